# revision 1
# baseline (speedup 1.0000x reference)
"""Trainium2 Bass kernel for a dense transformer block (B=2, T=2048, C=1024,
16 heads, causal attention with x64 score scale, MLP 4x), distributed over
8 NeuronCores.

Sharding: token-parallel.  Cores 0-3 take batch element 0, cores 4-7 batch
element 1.  Within a batch element the 16 query tiles of 128 tokens are dealt
round-robin (core j gets tiles j, j+4, j+8, j+12), which balances causal
attention cost and keeps the instruction stream identical across cores (SPMD):
core-dependent causal boundaries are handled with host-computed additive
masks.  K/V are computed redundantly per core for its whole batch element
(no collectives).

Precision: all matmuls run in float32r (fp32 with ~11-bit mantissa, full PE
rate for moving dim >= 256) except the attention-value product which runs in
bf16.  LayerNorm statistics are computed via ones-vector matmuls on the
partition (channel) axis since activations live transposed ([C, T]) on chip.
"""
import numpy as np

import concourse.bass as bass
import concourse.mybir as mybir
import concourse.tile as tile
from concourse.masks import make_identity
from concourse.vector_clock import ScopedClock
from concourse import bass_utils
from concourse.bass_utils import run_bass_kernel_spmd

_orig_run_command = bass_utils.run_command


def _run_command_ldwopt(cmd, **kw):
    return _orig_run_command(cmd, **kw)


P = 128
B, T, C = 2, 2048, 1024
NH, HD = 16, 64
NCT = C // P          # 8 channel tiles
NTC = T // 512        # 4 token 512-chunks per batch element
TOWN = 512            # own query tokens per core
NQT = TOWN // P       # 4 own query tiles
NG = NH // 2          # 8 head pairs
LN_EPS = 1e-5
FP = mybir.dt.float32
FR = mybir.dt.float32r
BF = mybir.dt.bfloat16
OP = mybir.AluOpType
AF = mybir.ActivationFunctionType
AX = mybir.AxisListType

# ---------------------------------------------------------------------------
# Workaround for walrus "Too many sync wait commands": most instruction
# structs in this compiler build accept only ~1 sync-wait.  Hoist overflow
# waits onto same-engine NoOps, and split the kernel-tail drain's
# global-clock waits across one drain instruction per clock domain.
# ---------------------------------------------------------------------------
_orig_commit_and_lower = tile.TileContext._commit_and_lower


def _ldw_sig(inst):
    a = inst.ins[0]
    try:
        return (a.memref, a.offset, str(a.ap), str(a.dtype))
    except AttributeError:
        return None


def _split_commit_and_lower(self, inst, original_block, old_bb_map, bb_to_exit_bb):
    # Drop weight reloads of the already-loaded stationary operand (the PE
    # queue executes in commit order; only Ldweights mutates the PE array).
    # Replace with a NoOp carrying the same sync info so semaphore counts
    # are unchanged.
    if isinstance(inst, mybir.InstLdweights):
        sig = _ldw_sig(inst)
        if sig is not None and sig == getattr(self, "_ldw_last_sig", None):
            si0 = inst.sync_info
            nop = mybir.InstNoOp(
                name=inst.name,
                sync_info=si0,
                bass_nofuse=True,
                engine=inst.engine,
            )
            return _split_commit_and_lower(self, nop, original_block,
                                           old_bb_map, bb_to_exit_bb)
        self._ldw_last_sig = sig
    si = getattr(inst, "sync_info", None)
    if (
        si is not None
        and si.on_wait
        and len(si.on_wait) > 1
        and type(inst).__name__.startswith("Inst")
    ):
        waits = list(si.on_wait)
        for w in waits[:-1]:
            nop = mybir.InstNoOp(
                name=self.nc.get_next_instruction_name(),
                sync_info=mybir.SyncInfo(on_wait=[w], on_update=[]),
                bass_nofuse=True,
                engine=inst.engine,
            )
            _orig_commit_and_lower(self, nop, original_block, old_bb_map, bb_to_exit_bb)
        inst.sync_info = mybir.SyncInfo(on_wait=waits[-1:], on_update=list(si.on_update))
    return _orig_commit_and_lower(self, inst, original_block, old_bb_map, bb_to_exit_bb)


def _split_drain_and_barrier(self, tick_clock, wait_clock):
    gc = tick_clock.global_clock
    entries = []
    scoped = gc.items() if hasattr(gc, "items") else [(None, gc)]
    for scope, vc in scoped:
        for proc in range(len(vc)):
            t = vc[proc]
            if t > 0:
                entries.append((scope, proc, t))
    if entries:
        for scope, proc, t in entries:
            drain_inst = self.nc.sync.drain()
            req = ScopedClock()
            req.require_at_least(scope, proc, t)
            wait_clock.add_sem_waits(drain_inst.ins, req)
    else:
        drain_inst = self.nc.sync.drain()
        wait_clock.add_sem_waits(
            drain_inst.ins, ScopedClock({None: tick_clock.global_clock})
        )
    self.nc.all_engine_barrier()
    assert self.sems is not None
    popped = self.nc._tile_sem_poison_stack.pop()
    assert popped is self._sem_poison
    self.nc.clear_and_free_semaphores(list(self.sems.allocated().values()))
    self.nc.all_engine_barrier()


def _apply_tile_patch():
    tile.TileContext._commit_and_lower = _split_commit_and_lower
    tile.TileContext._drain_and_barrier = _split_drain_and_barrier
    bass_utils.run_command = _run_command_ldwopt


# ---------------------------------------------------------------------------
# Host-side helpers
# ---------------------------------------------------------------------------

def _r12(a):
    """Round fp32 to float32r's grid (~11 mantissa bits) so on-device fp32r
    consumers see exactly representable values."""
    u = np.ascontiguousarray(a, np.float32).view(np.uint32).astype(np.uint64)
    u = (u + np.uint64(1 << 11)) & np.uint64(0xFFFFF000)
    return (u & np.uint64(0xFFFFFFFF)).astype(np.uint32).view(np.float32)


def _lhsT_tiles(w, km, mm):
    """[K, M] weight -> [M/128, K/128, 128, 128] lhsT tiles (w[m][k] block)."""
    k, m = w.shape
    return np.ascontiguousarray(
        w.reshape(km, P, mm, P).transpose(2, 0, 1, 3)
    )


# ---------------------------------------------------------------------------
# Device kernel builder
# ---------------------------------------------------------------------------

def _build(nc):
    xT = nc.dram_tensor("xT", [C, T], FP, kind="ExternalInput").ap()
    xTo = nc.dram_tensor("xTo", [C, TOWN], FP, kind="ExternalInput").ap()
    wq = nc.dram_tensor("wq", [NCT, NCT, P, P], FP, kind="ExternalInput").ap()
    wk = nc.dram_tensor("wk", [NCT, NCT, P, P], FP, kind="ExternalInput").ap()
    wv = nc.dram_tensor("wv", [NCT, NCT, P, P], FP, kind="ExternalInput").ap()
    wo = nc.dram_tensor("wo", [NCT, NCT, P, P], FP, kind="ExternalInput").ap()
    w1 = nc.dram_tensor("w1", [32, NCT, P, P], FP, kind="ExternalInput").ap()
    w2 = nc.dram_tensor("w2", [NCT, 32, P, P], FP, kind="ExternalInput").ap()
    gb = nc.dram_tensor("gb", [P, NCT, 4], FP, kind="ExternalInput").ap()
    msk = nc.dram_tensor("msk", [NQT, P, 512], FP, kind="ExternalInput").ap()
    outT = nc.dram_tensor("outT", [C, TOWN], FP, kind="ExternalOutput").ap()
    kscr = nc.dram_tensor("kscr", [C, T], FP, kind="ExternalOutput").ap()
    vscr = nc.dram_tensor("vscr", [T, C], BF, kind="ExternalOutput").ap()


    with tile.TileContext(nc) as tc:
        _build_tc(nc, tc, xT, xTo, wq, wk, wv, wo, w1, w2, gb, msk, outT,
                  kscr, vscr)
    return nc


def _layernorm_T(nc, tc, const, psum_st, psum_bc, ln_sb, src, dst, nchunks,
                 g_col, b_col, ones_col, ones_row, eps_t, tag, nb=2):
    """LayerNorm over the partition(channel) axis of src [128, NCT, nchunks*512]
    (float32r), writing normalized float32r to dst (may alias src).
    Statistics per token via ones-vector matmuls."""
    for ch in range(nchunks):
        sl = slice(ch * 512, (ch + 1) * 512)
        ssum = psum_st.tile([1, 512], FP, tag="ssum", bufs=1)
        ssq = psum_st.tile([1, 512], FP, tag="ssq", bufs=1)
        for ct in range(NCT):
            nc.tensor.matmul(ssum[:], ones_col[:], src[:, ct, sl],
                             start=(ct == 0), stop=(ct == NCT - 1))
        for ct in range(NCT):
            sq = ln_sb.tile([P, 512], FR, tag="sq", bufs=nb)
            nc.vector.tensor_tensor(sq[:], src[:, ct, sl], src[:, ct, sl], op=OP.mult)
            nc.tensor.matmul(ssq[:], ones_col[:], sq[:],
                             start=(ct == 0), stop=(ct == NCT - 1))
        mean = ln_sb.tile([1, 512], FP, tag="mean", bufs=nb)
        msq = ln_sb.tile([1, 512], FP, tag="msq", bufs=nb)
        nc.scalar.mul(mean[:], ssum[:], 1.0 / C)
        nc.scalar.mul(msq[:], ssq[:], 1.0 / C)
        var = ln_sb.tile([1, 512], FP, tag="var", bufs=nb)
        nc.vector.tensor_tensor(var[:], mean[:], mean[:], op=OP.mult)
        nc.vector.tensor_tensor(var[:], msq[:], var[:], op=OP.subtract)
        sd = ln_sb.tile([1, 512], FP, tag="sd", bufs=nb)
        nc.scalar.activation(sd[:], var[:], AF.Sqrt, bias=eps_t[0:1, :])
        rstd = ln_sb.tile([1, 512], FR, tag="rstd", bufs=nb)
        rstd_f = ln_sb.tile([1, 512], FP, tag="rstdf", bufs=nb)
        nc.vector.reciprocal(rstd_f[:], sd[:])
        nc.vector.tensor_copy(rstd[:], rstd_f[:])
        mrstd = ln_sb.tile([1, 512], FR, tag="mrstd", bufs=nb)
        nc.vector.tensor_tensor(mrstd[:], mean[:], rstd_f[:], op=OP.mult)
        # broadcast rstd and mean*rstd over all 128 partitions via outer product
        rb_ps = psum_bc.tile([P, 512], FP, tag="bcps", bufs=1, name="rb_ps")
        nc.tensor.matmul(rb_ps[:], ones_row[:], rstd[:], start=True, stop=True)
        rb = ln_sb.tile([P, 512], FP, tag="rb", bufs=nb)
        nc.scalar.copy(rb[:], rb_ps[:])
        mb_ps = psum_bc.tile([P, 512], FP, tag="bcps", bufs=1, name="mb_ps")
        nc.tensor.matmul(mb_ps[:], ones_row[:], mrstd[:], start=True, stop=True)
        mb = ln_sb.tile([P, 512], FP, tag="mb", bufs=nb)
        nc.scalar.copy(mb[:], mb_ps[:])
        for ct in range(NCT):
            t1 = ln_sb.tile([P, 512], FP, tag="t1", bufs=nb)
            nc.vector.tensor_tensor(t1[:], src[:, ct, sl], rb[:], op=OP.mult)
            t2 = ln_sb.tile([P, 512], FP, tag="t2", bufs=nb)
            nc.vector.tensor_tensor(t2[:], t1[:], mb[:], op=OP.subtract)
            nc.vector.tensor_scalar(
                dst[:, ct, sl], t2[:], g_col[:, ct:ct + 1], b_col[:, ct:ct + 1],
                op0=OP.mult, op1=OP.add,
            )


def _build_tc(nc, tc, xT, xTo, wq, wk, wv, wo, w1, w2, gb, msk, outT,
              kscr, vscr):
    const_cm = tc.tile_pool(name="const", bufs=1)
    const = const_cm.__enter__()
    ident = const.tile([P, P], BF)
    make_identity(nc, ident[:])
    ones_col = const.tile([P, 1], FR)
    nc.any.memset(ones_col[:].bitcast(FP), 1.0)
    ones_row = const.tile([1, P], FR)
    nc.any.memset(ones_row[:].bitcast(FP), 1.0)
    zeros512 = const.tile([P, 512], FP)
    nc.any.memset(zeros512[:], 0.0)
    eps_t = const.tile([P, 1], FP)
    nc.any.memset(eps_t[:], LN_EPS)
    gb_t = const.tile([P, NCT, 4], FP)
    nc.sync.dma_start(gb_t[:], gb)
    mask_t = const.tile([P, NQT, 512], FP)
    nc.sync.dma_start(mask_t[:], msk.rearrange("i p m -> p i m"))

    pers_cm = tc.tile_pool(name="pers", bufs=1)
    pers = pers_cm.__enter__()
    xn_own = pers.tile([P, NCT, TOWN], FR)     # 2 MB, lives to phase C
    out_t = pers.tile([P, NCT, TOWN], FR)      # 2 MB, phase B -> C

    g1c, b1c = gb_t[:, :, 0], gb_t[:, :, 1]
    g2c, b2c = gb_t[:, :, 2], gb_t[:, :, 3]

    poolAB_cm = tc.tile_pool(name="poolAB", bufs=1)
    poolAB = poolAB_cm.__enter__()
    q_t = poolAB.tile([P, NCT, TOWN], FR)      # 2 MB, phase A -> B

    # ---------------- Phase A: LN1 + K/V/Q projections -------------------
    with tc.tile_pool(name="xnpool", bufs=1) as xnpool, \
         tc.tile_pool(name="ln_sb", bufs=1) as ln_sb, \
         tc.tile_pool(name="wpool", bufs=1) as wpool, \
         tc.tile_pool(name="stage", bufs=3) as stage, \
         tc.tile_pool(name="psA", bufs=2, space="PSUM") as psA, \
         tc.tile_pool(name="psA_st", bufs=1, space="PSUM") as psA_st, \
         tc.tile_pool(name="psA_bc", bufs=1, space="PSUM") as psA_bc:
        xn_t = xnpool.tile([P, NCT, T], FR)    # 8 MB: x^T, overwritten by xn^T
        nc.sync.dma_start(xn_t[:], xT.rearrange("(ct p) t -> p ct t", p=P).bitcast(FR))
        nc.sync.dma_start(xn_own[:], xTo.rearrange("(ct p) t -> p ct t", p=P).bitcast(FR))

        _layernorm_T(nc, tc, const, psA_st, psA_bc, ln_sb, xn_t, xn_t, NTC,
                     g1c, b1c, ones_col, ones_row, eps_t, "ln1")
        _layernorm_T(nc, tc, const, psA_st, psA_bc, ln_sb, xn_own, xn_own, 1,
                     g1c, b1c, ones_col, ones_row, eps_t, "ln1o")

        # K^T = Wk^T @ xn^T -> kscr (weight-stationary, chunks inner)
        for m in range(NCT):
            wk_t = wpool.tile([P, NCT, P], FR, tag="wkt", bufs=2)
            nc.sync.dma_start(wk_t[:], wk[m].rearrange("k p m -> p k m").bitcast(FR))
            pss = [psA.tile([P, 512], FP, tag=f"pp{ch}", bufs=1, name=f"ps{ch}")
                   for ch in range(NTC)]
            for k in range(NCT):
                for ch in range(NTC):
                    nc.tensor.matmul(pss[ch][:], wk_t[:, k],
                                     xn_t[:, k, ch * 512:(ch + 1) * 512],
                                     start=(k == 0), stop=(k == NCT - 1))
            for ch in range(NTC):
                st = stage.tile([P, 512], FR, tag="kst")
                nc.vector.tensor_copy(st[:], pss[ch][:])
                nc.sync.dma_start(kscr[m * P:(m + 1) * P, ch * 512:(ch + 1) * 512],
                                  st[:].bitcast(FP))

        # V^T = Wv^T @ xn^T, then PE-transpose blocks -> token-major vscr
        for m in range(NCT):
            wv_t = wpool.tile([P, NCT, P], FR, tag="wkt", bufs=2, name="wv_t")
            nc.sync.dma_start(wv_t[:], wv[m].rearrange("k p m -> p k m").bitcast(FR))
            pss = [psA.tile([P, 512], FP, tag=f"pp{ch}", bufs=1, name=f"ps{ch}")
                   for ch in range(NTC)]
            for k in range(NCT):
                for ch in range(NTC):
                    nc.tensor.matmul(pss[ch][:], wv_t[:, k],
                                     xn_t[:, k, ch * 512:(ch + 1) * 512],
                                     start=(k == 0), stop=(k == NCT - 1))
            for ch in range(NTC):
                st = stage.tile([P, 512], BF, tag="vst")
                nc.vector.tensor_copy(st[:], pss[ch][:])
                pst = psA_st.tile([P, 512], BF, tag="vtp", bufs=1, name="pst")
                for j in range(4):
                    nc.tensor.transpose(pst[:, j * P:(j + 1) * P],
                                        st[:, j * P:(j + 1) * P], ident[:])
                stv = stage.tile([P, 512], BF, tag="vstT", name="stv")
                nc.scalar.copy(stv[:], pst[:])
                for j in range(4):
                    nc.sync.dma_start(
                        vscr[ch * 512 + j * P:ch * 512 + (j + 1) * P,
                             m * P:(m + 1) * P],
                        stv[:, j * P:(j + 1) * P])

        # Q^T (own tokens), folded x64 score scale
        for m in range(NCT):
            wq_t = wpool.tile([P, NCT, P], FR, tag="wkt", bufs=2)
            nc.sync.dma_start(wq_t[:], wq[m].rearrange("k p m -> p k m").bitcast(FR))
            ps = psA.tile([P, 512], FP, tag="pp0", bufs=1)
            for k in range(NCT):
                nc.tensor.matmul(ps[:], wq_t[:, k], xn_own[:, k, :],
                                 start=(k == 0), stop=(k == NCT - 1))
            nc.vector.tensor_scalar_mul(q_t[:, m], ps[:], float(HD))

    # ---------------- Phase B: attention ---------------------------------
    with tc.tile_pool(name="kvpool", bufs=2) as kvpool, \
         tc.tile_pool(name="attpool", bufs=2) as attpool, \
         tc.tile_pool(name="attsm", bufs=3) as attsm, \
         tc.tile_pool(name="psB_s", bufs=4, space="PSUM") as psB_s, \
         tc.tile_pool(name="psB_t", bufs=2, space="PSUM") as psB_t, \
         tc.tile_pool(name="psB_o", bufs=1, space="PSUM") as psB_o:
        for g in range(NG):
            k_g = kvpool.tile([P, T], FR, tag="kg")
            nc.sync.dma_start(k_g[:], kscr[g * P:(g + 1) * P, :].bitcast(FR))
            v_g = kvpool.tile([P, T // P, P], BF, tag="vg")
            nc.sync.dma_start(v_g[:], vscr[:, g * P:(g + 1) * P]
                              .rearrange("(n p) d -> p n d", p=P))
            for i in range(NQT):
                nch = i + 1
                E = nch * 512
                ps_av = [psB_o.tile([P, P], FP, tag=f"ops{h2}", name=f"ps_av{h2}")
                         for h2 in range(2)]
                attTs = []
                for h2 in range(2):
                    pb = h2 * 64
                    q_sl = q_t[pb:pb + 64, g, i * P:(i + 1) * P]
                    sc = attpool.tile([P, T], FP, tag="scs")
                    mx = attsm.tile([P, NQT], FP, tag="mx")
                    for kk in range(nch):
                        ps_s = psB_s.tile([P, 512], FP, tag="sps")
                        nc.tensor.matmul(ps_s[:], q_sl,
                                         k_g[pb:pb + 64, kk * 512:(kk + 1) * 512],
                                         start=True, stop=True)
                        sc_chunk = sc[:, kk * 512:(kk + 1) * 512]
                        if kk == i:
                            nc.vector.tensor_tensor(sc_chunk, ps_s[:],
                                                    mask_t[:, i, :], op=OP.add)
                            nc.vector.tensor_reduce(mx[:, kk:kk + 1], sc_chunk,
                                                    axis=AX.X, op=OP.max)
                        else:
                            nc.scalar.copy(sc_chunk, ps_s[:])
                            nc.vector.tensor_reduce(mx[:, kk:kk + 1], ps_s[:],
                                                    axis=AX.X, op=OP.max)
                    nm = attsm.tile([P, 1], FP, tag="nm")
                    nc.vector.tensor_reduce(nm[:], mx[:, 0:nch], axis=AX.X, op=OP.max)
                    nc.vector.tensor_scalar_mul(nm[:], nm[:], -1.0)
                    att = attpool.tile([P, T], BF, tag="att")
                    dsum = attsm.tile([P, NQT], FP, tag="dsum")
                    for kk in range(nch):
                        nc.scalar.activation(
                            att[:, kk * 512:(kk + 1) * 512],
                            sc[:, kk * 512:(kk + 1) * 512],
                            AF.Exp, bias=nm[:], accum_out=dsum[:, kk:kk + 1])
                    den = attsm.tile([P, 1], FP, tag="den")
                    nc.vector.tensor_reduce(den[:], dsum[:, 0:nch], axis=AX.X, op=OP.add)
                    rden = attsm.tile([P, 1], FP, tag="rden")
                    nc.vector.reciprocal(rden[:], den[:])
                    attn = attpool.tile([P, E], BF, tag=f"attn{i}", name=f"attn{i}")
                    nc.vector.tensor_scalar_mul(attn[:], att[:, 0:E], rden[:])
                    attT = attpool.tile([P, nch * 4, P], BF, tag=f"attT{h2}_{i}",
                                        name=f"attT{h2}_{i}")
                    for kk in range(nch):
                        ps_t = psB_t.tile([P, 512], BF, tag="tps")
                        for b4 in range(4):
                            blk = kk * 4 + b4
                            nc.tensor.transpose(ps_t[:, b4 * P:(b4 + 1) * P],
                                                attn[:, blk * P:(blk + 1) * P],
                                                ident[:])
                        ev = attT[:, kk * 4:(kk + 1) * 4, :].rearrange("p n d -> p (n d)")
                        if kk % 2 == 0:
                            nc.scalar.copy(ev, ps_t[:])
                        else:
                            nc.vector.tensor_copy(ev, ps_t[:])
                    attTs.append(attT)
                # av: shared full-width v block as stationary; each head's
                # valid half lands on its own partition range
                for blk in range(nch * 4):
                    for h2 in range(2):
                        nc.tensor.matmul(
                            ps_av[h2][:], v_g[:, blk, :], attTs[h2][:, blk, :],
                            start=(blk == 0), stop=(blk == nch * 4 - 1))
                nc.vector.tensor_copy(out_t[0:64, g, i * P:(i + 1) * P],
                                      ps_av[0][0:64, :])
                nc.vector.tensor_copy(out_t[64:128, g, i * P:(i + 1) * P],
                                      ps_av[1][64:128, :])

    poolAB_cm.__exit__(None, None, None)

    # ---------------- Phase C: out-proj, LN2, MLP ------------------------
    with tc.tile_pool(name="wpoolC", bufs=1) as wpoolC, \
         tc.tile_pool(name="ln_sbC", bufs=1) as ln_sbC, \
         tc.tile_pool(name="apool", bufs=1) as apool, \
         tc.tile_pool(name="hpool", bufs=1) as hpool, \
         tc.tile_pool(name="opool", bufs=2) as opool, \
         tc.tile_pool(name="psC", bufs=3, space="PSUM") as psC, \
         tc.tile_pool(name="psC_st", bufs=1, space="PSUM") as psC_st, \
         tc.tile_pool(name="psC_bc", bufs=1, space="PSUM") as psC_bc:
        h_t = hpool.tile([P, NCT, TOWN], FR)
        h2_t = hpool.tile([P, NCT, TOWN], FR)
        for m in range(NCT):
            wo_t = wpoolC.tile([P, NCT, P], FR, tag="wot", bufs=2)
            nc.sync.dma_start(wo_t[:], wo[m].rearrange("k p m -> p k m").bitcast(FR))
            ps = psC.tile([P, 512], FP, tag="psC", bufs=3)
            for k in range(NCT):
                nc.tensor.matmul(ps[:], wo_t[:, k], out_t[:, k, :],
                                 start=(k == 0), stop=(k == NCT - 1))
            nc.vector.tensor_tensor(h_t[:, m], ps[:], xn_own[:, m], op=OP.add)

        _layernorm_T(nc, tc, const, psC_st, psC_bc, ln_sbC, h_t, h2_t, 1,
                     g2c, b2c, ones_col, ones_row, eps_t, "ln2", nb=1)

        a_t = apool.tile([P, 32, TOWN], FR)    # 8 MB
        for m in range(32):
            w1_t = wpoolC.tile([P, NCT, P], FR, tag="wot", bufs=2)
            nc.sync.dma_start(w1_t[:], w1[m].rearrange("k p m -> p k m").bitcast(FR))
            ps = psC.tile([P, 512], FP, tag="psC", bufs=3)
            for k in range(NCT):
                nc.tensor.matmul(ps[:], w1_t[:, k], h2_t[:, k, :],
                                 start=(k == 0), stop=(k == NCT - 1))
            nc.vector.tensor_scalar_max(a_t[:, m], ps[:], 0.0)

        for m in range(NCT):
            w2_t = wpoolC.tile([P, 32, P], FR, tag="w2t", bufs=2)
            nc.sync.dma_start(w2_t[:], w2[m].rearrange("k p m -> p k m").bitcast(FR))
            ps = psC.tile([P, 512], FP, tag="psC", bufs=3)
            for k in range(32):
                nc.tensor.matmul(ps[:], w2_t[:, k], a_t[:, k, :],
                                 start=(k == 0), stop=(k == 31))
            o_m = opool.tile([P, 512], FP, tag="om")
            nc.vector.tensor_tensor(o_m[:], ps[:], h2_t[:, m], op=OP.add)
            nc.sync.dma_start(outT[m * P:(m + 1) * P, :], o_m[:])

    pers_cm.__exit__(None, None, None)
    const_cm.__exit__(None, None, None)


# ---------------------------------------------------------------------------
# Public entry point
# ---------------------------------------------------------------------------
_cache = {}


def _get_nc():
    if "nc" not in _cache:
        _apply_tile_patch()
        nc = bass.Bass("TRN2", target_bir_lowering=False, debug=False,
                       num_devices=8)
        _build(nc)
        _cache["nc"] = nc
    return _cache["nc"]


def run(inputs, trace=False):
    x = np.asarray(inputs["x"], np.float32)
    Wk = np.asarray(inputs["Wk"], np.float32)
    Wq = np.asarray(inputs["Wq"], np.float32)
    Wv = np.asarray(inputs["Wv"], np.float32)
    Wo = np.asarray(inputs["Wo"], np.float32)
    W1 = np.asarray(inputs["W1"], np.float32)
    W2 = np.asarray(inputs["W2"], np.float32)
    g1 = np.asarray(inputs["g1"], np.float32)
    b1 = np.asarray(inputs["b1"], np.float32)
    g2 = np.asarray(inputs["g2"], np.float32)
    b2 = np.asarray(inputs["b2"], np.float32)

    wq_t = _r12(_lhsT_tiles(Wq, NCT, NCT))
    wk_t = _r12(_lhsT_tiles(Wk, NCT, NCT))
    wo_t = _r12(_lhsT_tiles(Wo, NCT, NCT))
    wv_t = _r12(_lhsT_tiles(Wv, NCT, NCT))
    w1_t = _r12(_lhsT_tiles(W1, NCT, 32))
    w2_t = _r12(_lhsT_tiles(W2, 32, NCT))
    gbh = np.stack(
        [g1.reshape(NCT, P).T, b1.reshape(NCT, P).T,
         g2.reshape(NCT, P).T, b2.reshape(NCT, P).T], axis=-1
    ).astype(np.float32)  # [P, NCT, 4]

    in_maps = []
    own_tokens_by_core = []
    for c in range(8):
        b = c // 4
        j = c % 4
        tiles = [j + 4 * i for i in range(NQT)]
        toks = np.concatenate([np.arange(t * P, (t + 1) * P) for t in tiles])
        own_tokens_by_core.append((b, toks))
        xT_full = _r12(np.ascontiguousarray(x[b].T))
        xT_own = _r12(np.ascontiguousarray(x[b][toks].T))
        mask = np.zeros((NQT, P, 512), np.float32)
        for i in range(NQT):
            t0 = (j + 4 * i) * P
            Ei = (i + 1) * 512
            cols = (Ei - 512) + np.arange(512)
            rows = t0 + np.arange(P)
            mask[i] = np.where(cols[None, :] <= rows[:, None], 0.0, -1.0e30)
        in_maps.append({
            "xT": xT_full, "xTo": xT_own,
            "wq": wq_t, "wk": wk_t, "wv": wv_t, "wo": wo_t,
            "w1": w1_t, "w2": w2_t, "gb": gbh, "msk": mask,
        })

    nc = _get_nc()
    res = run_bass_kernel_spmd(nc, in_maps, core_ids=list(range(8)),
                               trace=trace)

    out = np.empty((B, T, C), np.float32)
    for c in range(8):
        b, toks = own_tokens_by_core[c]
        out[b, toks, :] = res.results[c]["outT"].T
    return out, res


def kernel(**inputs):
    out, _ = run(inputs, trace=False)
    return out

